# revision 5
# baseline (speedup 1.0000x reference)
"""Multi-head attention (ALiBi, symmetric) Trainium2 kernel.

Problem: B=2, L=2048, C=1024, H=16 heads, D=64 (torch-Linear projections,
symmetric ALiBi bias, softmax, output projection + bias).

Sharding: 8 cores = batch (2) x head-group (4). Head groups take one head
from each "slope tier" so banded-attention work is balanced:
    g0=[15,9,7,3], g1=[14,8,6,2], g2=[13,10,5,1], g3=[12,11,4,0]
Each core computes its 4 heads' attention and a partial output projection
(columns of Wo restricted to its heads); the host sums the 4 partials per
batch and adds bo. No on-device collectives.

Key tricks:
  - softmax without max-subtraction (scores bounded: qk/8 ~ +-3, bias <= 0)
  - ALiBi bias applied INSIDE the QK^T matmul via augmentation rows:
    bias = +-(s*j - s*i) encoded with 11 exact bf16 bit-planes of j (and of
    i), ones rows on the opposite operand. Two kh variants (lower/upper
    triangle); diagonal 128x128 chunks use a base matmul + DVE add of a
    constant bias tile.
  - S^T layout ([keys, queries]) so the softmax denominator comes free from
    a ones-column appended to V in the PV matmul, and attn^T feeds both the
    PV and output-projection matmuls with no transposes.
  - per-query normalization folded into the PV PSUM->SBUF copy (multiply by
    DMA-broadcast reciprocal of the denominator row).
  - banded attention: tiles with min-distance bias < -7 are skipped
    (exp < 1e-3 relative; verified 4.5e-3 rel err vs fp32 reference).

All matmuls bf16 inputs / fp32 PSUM accumulation.
"""

import os
import numpy as np
import ml_dtypes

import concourse.bass as bass
import concourse.tile as tile
from concourse import bacc, mybir

H = 16
D = 64
L = 2048
C = 1024
NB = 11            # bit planes for j/i (2048 = 2^11)
BAND_T = 7.0       # drop tiles with slope*dist > BAND_T
JT = 128           # key tile (partitions)
IT = 512           # query tile (matmul N)
PAIR = 2 * IT      # exp processed at [128, 1024]
N_CORES = 8
GROUPS = [[15, 9, 7, 3], [14, 8, 6, 2], [13, 10, 5, 1], [12, 11, 4, 0]]
F32 = mybir.dt.float32
BF16 = mybir.dt.bfloat16
BF16_NP = ml_dtypes.bfloat16

_last_results = None


def _slopes():
    start = 2.0 ** (-(2.0 ** -(np.log2(H) - 3)))
    return np.array([start * (start ** i) for i in range(H)], dtype=np.float32)


SLOPES = _slopes()
# slot s keeps the union band over slope tier s (slot s of group g holds
# head GROUPS[g][s]; tiers are {12..15},{8..11},{4..7},{0..3} and the widest
# band in tier s is the smallest slope = SLOPES[tier_max_head]).
TIER_SLOPE = [SLOPES[15], SLOPES[11], SLOPES[7], SLOPES[3]]


def _keep(s, Jt, It):
    """slot s, key tile Jt (128-wide), query tile It (512-wide)."""
    J0, I0 = Jt * JT, It * IT
    dist = max(0, max(J0 - (I0 + IT - 1), I0 - (J0 + JT - 1)))
    return TIER_SLOPE[s] * dist <= BAND_T


def build_bass():
    nc = bacc.Bacc("TRN2", target_bir_lowering=False, debug=False,
                   num_devices=N_CORES)

    xT_d = nc.dram_tensor("xT", [C, L], BF16, kind="ExternalInput")
    wqT_d = nc.dram_tensor("wqT", [C, 4 * D], BF16, kind="ExternalInput")
    wkT_d = nc.dram_tensor("wkT", [C, 4 * D], BF16, kind="ExternalInput")
    wvT_d = nc.dram_tensor("wvT", [C, 4 * D], BF16, kind="ExternalInput")
    woT_d = nc.dram_tensor("woT", [2, 128, C], BF16, kind="ExternalInput")
    augq_d = nc.dram_tensor("augq", [4, 2 * NB, L], BF16, kind="ExternalInput")
    augkl_d = nc.dram_tensor("augkl", [4, 2 * NB, L], BF16, kind="ExternalInput")
    augku_d = nc.dram_tensor("augku", [4, 2 * NB, L], BF16, kind="ExternalInput")
    tdiag_d = nc.dram_tensor("tdiag", [4, JT, JT], F32, kind="ExternalInput")
    out_d = nc.dram_tensor("out", [L, C], F32, kind="ExternalOutput")

    KT = C // 128  # 8 contraction tiles
    AUGP = D + 2 * NB  # 86 partitions for augmented qk matmuls

    with tile.TileContext(nc) as tc:
        with (
            tc.tile_pool(name="const", bufs=1) as const,
            tc.tile_pool(name="psm", bufs=3, space="PSUM") as psm,
            tc.tile_pool(name="pspv", bufs=2, space="PSUM") as pspv,
            tc.tile_pool(name="ppool", bufs=6) as ppool,
            tc.tile_pool(name="ypool", bufs=4) as ypool,
            tc.tile_pool(name="rpool", bufs=3) as rpool,
            tc.tile_pool(name="dpool", bufs=3, space="DRAM") as dpool,
        ):
            # ---- load inputs ----
            xT_sb = const.tile([128, KT, L], BF16)
            for kt in range(KT):
                nc.sync.dma_start(out=xT_sb[:, kt, :],
                                  in_=xT_d.ap()[kt * 128:(kt + 1) * 128, :])
            wqT_sb = const.tile([128, KT, 4 * D], BF16)
            wkT_sb = const.tile([128, KT, 4 * D], BF16)
            wvT_sb = const.tile([128, KT, 4 * D], BF16)
            for w_sb, w_d in ((wqT_sb, wqT_d), (wkT_sb, wkT_d), (wvT_sb, wvT_d)):
                for kt in range(KT):
                    nc.sync.dma_start(out=w_sb[:, kt, :],
                                      in_=w_d.ap()[kt * 128:(kt + 1) * 128, :])
            woT_sb = const.tile([128, 2, C], BF16)
            for p in range(2):
                nc.sync.dma_start(out=woT_sb[:, p, :], in_=woT_d.ap()[p])
            tdiag_sb = const.tile([128, 4, JT], F32)
            for s in range(4):
                nc.sync.dma_start(out=tdiag_sb[:, s, :], in_=tdiag_d.ap()[s])

            qaug_sb = []
            khl_sb = []
            khu_sb = []
            for s in range(4):
                qa = const.tile([AUGP, L], BF16, tag=f"qaug{s}")
                kl = const.tile([AUGP, L], BF16, tag=f"khl{s}")
                ku = const.tile([AUGP, L], BF16, tag=f"khu{s}")
                nc.sync.dma_start(out=qa[D:AUGP, :], in_=augq_d.ap()[s])
                nc.sync.dma_start(out=kl[D:AUGP, :], in_=augkl_d.ap()[s])
                nc.sync.dma_start(out=ku[D:AUGP, :], in_=augku_d.ap()[s])
                qaug_sb.append(qa)
                khl_sb.append(kl)
                khu_sb.append(ku)

            # V with ones column appended (PV ones-row => softmax denominator)
            vh_sb = const.tile([128, L // JT, 4, D + 1], BF16)
            nc.vector.memset(vh_sb[:, :, :, D:D + 1], 1.0)

            outT_sb = [const.tile([128, L], BF16, tag=f"outT{p}",
                                  name=f"outT{p}") for p in range(2)]

            # ---- projections ----
            # q^T, k^T in [channel, L] layout  (2 slots per 128-chan tile)
            for ct in range(2):
                for lt in range(L // IT):
                    psq = psm.tile([128, IT], F32, tag="ps")
                    psk = psm.tile([128, IT], F32, tag="ps")
                    for kt in range(KT):
                        nc.tensor.matmul(
                            psq, lhsT=wqT_sb[:, kt, ct * 128:(ct + 1) * 128],
                            rhs=xT_sb[:, kt, lt * IT:(lt + 1) * IT],
                            start=(kt == 0), stop=(kt == KT - 1))
                    for kt in range(KT):
                        nc.tensor.matmul(
                            psk, lhsT=wkT_sb[:, kt, ct * 128:(ct + 1) * 128],
                            rhs=xT_sb[:, kt, lt * IT:(lt + 1) * IT],
                            start=(kt == 0), stop=(kt == KT - 1))
                    for half in range(2):
                        s = ct * 2 + half
                        sl = slice(half * D, (half + 1) * D)
                        dst = slice(lt * IT, (lt + 1) * IT)
                        nc.vector.tensor_copy(out=qaug_sb[s][0:D, dst],
                                              in_=psq[sl, :])
                        nc.vector.tensor_copy(out=khl_sb[s][0:D, dst],
                                              in_=psk[sl, :])
                        nc.vector.tensor_copy(out=khu_sb[s][0:D, dst],
                                              in_=psk[sl, :])
            # v in [L, channel] layout
            for jt in range(L // JT):
                psv = psm.tile([128, 4 * D], F32, tag="ps")
                for kt in range(KT):
                    nc.tensor.matmul(
                        psv, lhsT=xT_sb[:, kt, jt * JT:(jt + 1) * JT],
                        rhs=wvT_sb[:, kt, :],
                        start=(kt == 0), stop=(kt == KT - 1))
                for s in range(4):
                    nc.vector.tensor_copy(
                        out=vh_sb[:, jt, s, 0:D],
                        in_=psv[:, s * D:(s + 1) * D])

            # ---- attention (banded), slot by slot ----
            for s in range(4):
                for m in range(L // PAIR):  # 2 query pairs of 1024
                    I_a, I_b = 2 * m, 2 * m + 1
                    base_i = m * PAIR
                    js_a = [j for j in range(L // JT) if _keep(s, j, I_a)]
                    js_b = [j for j in range(L // JT) if _keep(s, j, I_b)]
                    js_all = [j for j in range(L // JT)
                              if j in js_a or j in js_b]
                    pv = {}
                    if js_a:
                        pv[I_a] = pspv.tile([D + 1, IT], F32, tag="pv",
                                            name=f"pv{s}_{I_a}")
                    if js_b:
                        pv[I_b] = pspv.tile([D + 1, IT], F32, tag="pv",
                                            name=f"pv{s}_{I_b}")
                    for j in js_all:
                        in_a, in_b = j in js_a, j in js_b
                        cl = 0 if in_a else IT          # kept col range lo
                        ch = PAIR if in_b else IT       # kept col range hi
                        st = psm.tile([128, PAIR], F32, tag="ps")
                        J0 = j * JT
                        if base_i <= J0 < base_i + PAIR:
                            # diagonal pair: 128-wide chunks by side
                            for c in range(cl // JT, ch // JT):
                                c0 = base_i + c * JT
                                cs = slice(c * JT, (c + 1) * JT)
                                qs = slice(c0, c0 + JT)
                                if c0 == J0:
                                    nc.tensor.matmul(
                                        st[:, cs],
                                        lhsT=khl_sb[s][0:D, J0:J0 + JT],
                                        rhs=qaug_sb[s][0:D, qs],
                                        start=True, stop=True)
                                    nc.vector.tensor_add(
                                        st[:, cs], st[:, cs], tdiag_sb[:, s, :])
                                else:
                                    kh = khl_sb[s] if c0 > J0 else khu_sb[s]
                                    nc.tensor.matmul(
                                        st[:, cs],
                                        lhsT=kh[:, J0:J0 + JT],
                                        rhs=qaug_sb[s][:, qs],
                                        start=True, stop=True)
                        else:
                            kh = khl_sb[s] if J0 < base_i else khu_sb[s]
                            for c in range(cl // IT, ch // IT):
                                cs = slice(c * IT, (c + 1) * IT)
                                qs = slice(base_i + c * IT,
                                           base_i + (c + 1) * IT)
                                nc.tensor.matmul(
                                    st[:, cs], lhsT=kh[:, J0:J0 + JT],
                                    rhs=qaug_sb[s][:, qs],
                                    start=True, stop=True)
                        p = ppool.tile([128, PAIR], BF16, tag="p")
                        nc.scalar.activation(out=p[:, cl:ch], in_=st[:, cl:ch],
                                             func=mybir.ActivationFunctionType.Exp)
                        for I, js in ((I_a, js_a), (I_b, js_b)):
                            if j in js:
                                half = (I % 2) * IT
                                nc.tensor.matmul(
                                    pv[I], lhsT=vh_sb[:, j, s, :],
                                    rhs=p[:, half:half + IT],
                                    start=(j == js[0]), stop=(j == js[-1]))
                    # normalize + write attn output (transposed layout)
                    for I in sorted(pv):
                        den = rpool.tile([1, IT], F32, tag="den")
                        nc.vector.tensor_copy(out=den, in_=pv[I][D:D + 1, :])
                        dbounce = dpool.tile([1, IT], F32, tag="dbounce",
                                             name=f"dbounce{s}_{I}")
                        nc.sync.dma_start(out=dbounce, in_=den)
                        rec = rpool.tile([D, IT], F32, tag="rec")
                        nc.sync.dma_start(out=rec,
                                          in_=dbounce.to_broadcast([D, IT]))
                        nc.vector.reciprocal(rec, rec)
                        dst_rows = slice((s % 2) * D, (s % 2) * D + D)
                        nc.vector.tensor_mul(
                            out=outT_sb[s // 2][dst_rows, I * IT:(I + 1) * IT],
                            in0=pv[I][0:D, :], in1=rec)

            # ---- output projection (contraction over 256 chans = 2 pairs) ----
            for lt in range(L // JT):
                ls = slice(lt * JT, (lt + 1) * JT)
                for ct in range(2):
                    cs = slice(ct * IT, (ct + 1) * IT)
                    psy = psm.tile([128, IT], F32, tag="ps")
                    nc.tensor.matmul(psy, lhsT=outT_sb[0][:, ls],
                                     rhs=woT_sb[:, 0, cs], start=True, stop=False)
                    nc.tensor.matmul(psy, lhsT=outT_sb[1][:, ls],
                                     rhs=woT_sb[:, 1, cs], start=False, stop=True)
                    y = ypool.tile([128, IT], F32, tag="y")
                    if ct == 0:
                        nc.vector.tensor_copy(out=y, in_=psy)
                    else:
                        nc.scalar.copy(out=y, in_=psy)
                    nc.sync.dma_start(out=out_d.ap()[ls, cs], in_=y)

    nc.compile()
    return nc


def _prep_core_inputs(q, Wq, Wk, Wv, Wo, b, g):
    heads = GROUPS[g]
    xT = np.ascontiguousarray(q[b].T).astype(BF16_NP)

    def stackT(W, scale=1.0):
        rows = np.concatenate([W[h * D:(h + 1) * D, :] for h in heads], axis=0)
        return np.ascontiguousarray(rows.T * scale).astype(BF16_NP)

    wqT = stackT(Wq, 1.0 / np.sqrt(D))
    wkT = stackT(Wk)
    wvT = stackT(Wv)
    woT = np.stack([
        np.concatenate([np.ascontiguousarray(Wo[:, h * D:(h + 1) * D].T)
                        for h in heads[2 * p:2 * p + 2]], axis=0)
        for p in range(2)]).astype(BF16_NP)

    jj = np.arange(L)
    bits = ((jj[None, :] >> np.arange(NB)[:, None]) & 1).astype(np.float32)
    ones = np.ones((NB, L), dtype=np.float32)
    augq = np.zeros((4, 2 * NB, L), dtype=np.float32)
    augkl = np.zeros((4, 2 * NB, L), dtype=np.float32)
    augku = np.zeros((4, 2 * NB, L), dtype=np.float32)
    tdiag = np.zeros((4, JT, JT), dtype=np.float32)
    for s, h in enumerate(heads):
        sb = float(np.float32(SLOPES[h]).astype(BF16_NP).astype(np.float32))
        planes = (bits * (2.0 ** np.arange(NB))[:, None] * sb).astype(BF16_NP)
        planes = planes.astype(np.float32)  # exact bf16 values
        augq[s] = np.concatenate([ones, planes], axis=0)
        augkl[s] = np.concatenate([planes, -ones], axis=0)
        augku[s] = np.concatenate([-planes, ones], axis=0)
        d = np.abs(jj[:JT][None, :] - jj[:JT][:, None]).astype(np.float32)
        tdiag[s] = -sb * d
    return {
        "xT": xT, "wqT": wqT, "wkT": wkT, "wvT": wvT, "woT": woT,
        "augq": augq.astype(BF16_NP), "augkl": augkl.astype(BF16_NP),
        "augku": augku.astype(BF16_NP), "tdiag": tdiag,
    }


def kernel(q, Wq, Wk, Wv, Wo, bo):
    global _last_results
    q = np.asarray(q, dtype=np.float32)
    Wq = np.asarray(Wq, dtype=np.float32)
    Wk = np.asarray(Wk, dtype=np.float32)
    Wv = np.asarray(Wv, dtype=np.float32)
    Wo = np.asarray(Wo, dtype=np.float32)
    bo = np.asarray(bo, dtype=np.float32)

    trace = bool(os.environ.get("BASS_TRACE"))
    if trace:
        _install_axon_prof_shim()
    from concourse.bass_utils import run_bass_kernel_spmd

    nc = build_bass()
    in_maps = [_prep_core_inputs(q, Wq, Wk, Wv, Wo, core // 4, core % 4)
               for core in range(N_CORES)]
    res = run_bass_kernel_spmd(nc, in_maps, core_ids=list(range(N_CORES)),
                               trace=trace)
    _last_results = res
    B = q.shape[0]
    out = np.zeros((B, L, C), dtype=np.float32)
    for core in range(N_CORES):
        out[core // 4] += res.results[core]["out"]
    out += bo[None, None, :]
    return out


def _install_axon_prof_shim():
    """Provide the missing antenv.axon_hooks so trace=True works under axon."""
    import contextlib
    import ctypes
    import sys
    import types

    if "antenv.axon_hooks" in sys.modules:
        return
    so_path = "/opt/axon/libaxon_pjrt.so"
    try:
        lib = ctypes.CDLL(so_path)
    except OSError:
        return
    if not hasattr(lib, "axon_start_nrt_profile"):
        return
    lib.axon_start_nrt_profile.argtypes = [ctypes.POINTER(ctypes.c_int64),
                                           ctypes.c_size_t]
    lib.axon_start_nrt_profile.restype = ctypes.c_int64
    lib.axon_stop_nrt_profile.argtypes = [ctypes.c_char_p]
    lib.axon_stop_nrt_profile.restype = ctypes.c_int64

    @contextlib.contextmanager
    def _hook(output_dir, device_ids):
        import jax
        jax.devices()
        if device_ids:
            ids = (ctypes.c_int64 * len(device_ids))(*device_ids)
            rc = lib.axon_start_nrt_profile(ids, len(device_ids))
        else:
            rc = lib.axon_start_nrt_profile(None, 0)
        if rc != 0:
            raise RuntimeError(f"axon_start_nrt_profile rc={rc}")
        try:
            yield
        finally:
            n = lib.axon_stop_nrt_profile(str(output_dir).encode())
            print(f"profile: {n} file(s) -> {output_dir}", file=sys.stderr)

    mod = types.ModuleType("antenv.axon_hooks")
    mod.get_axon_ntff_profile_hook = lambda: _hook
    mod.set_axon_ntff_profile_hook = lambda h: None
    sys.modules["antenv.axon_hooks"] = mod
    try:
        import antenv
        antenv.axon_hooks = mod
    except ImportError:
        pass
